# revision 15
# baseline (speedup 1.0000x reference)
"""Trainium2 Bass kernel for nn_DE_GCL_vel_2D (GNN message passing, E(8e5) edges, N(5e4) nodes).

Strategy: sort edges by destination node (`row`), partition nodes into 8
contiguous ranges with ~equal edge counts -> each core's segment-sum is
purely local (no collective). Within a core, nodes are packed into blocks
of <=127 real nodes / <=2048 edges (slot 127 is a trash slot for padding
edges). Edge MLP runs feature-major on the PE; segment-sum is done with
one-hot matmuls accumulating into a per-block PSUM tile.
"""
import numpy as np
from contextlib import ExitStack

N = 50000
E = 800000
NF = 128
H = 128
NCORES = 8
TILE = 512           # edges per tile
TPB = 4              # tiles per block
BLK_E = TILE * TPB   # 2048 edges capacity per block
BLK_N = 127          # max real nodes per block (slot 127 = trash)


def _host_prep(h, coord, vel, edge_index):
    """Shard + marshal inputs. Returns (per_core_inputs, metas, B)."""
    row = np.asarray(edge_index[0]).astype(np.int64)
    col = np.asarray(edge_index[1]).astype(np.int64)
    n = h.shape[0]
    order = np.argsort(row, kind="stable")
    row_s = row[order]
    col_s = col[order]
    deg = np.bincount(row, minlength=n)
    cum = np.zeros(n + 1, np.int64)
    np.cumsum(deg, out=cum[1:])
    ne_total = row.shape[0]
    targets = (np.arange(1, NCORES) * ne_total) // NCORES
    inner = np.searchsorted(cum, targets)
    bounds = np.concatenate(([0], inner, [n])).astype(np.int64)
    assert deg.max() <= BLK_E, "node degree exceeds block capacity"

    cores_blocks = []
    for c in range(NCORES):
        n0, n1 = int(bounds[c]), int(bounds[c + 1])
        blocks = []
        nb = n0
        while nb < n1:
            base = nb
            ne = 0
            nn = 0
            while nb < n1 and nn < BLK_N and ne + deg[nb] <= BLK_E:
                ne += int(deg[nb])
                nn += 1
                nb += 1
            blocks.append((base, nn, int(cum[base]), ne))
        cores_blocks.append(blocks)
    B = max(len(bl) for bl in cores_blocks)

    per_core = []
    metas = []
    for c in range(NCORES):
        blocks = cores_blocks[c]
        node_idx = np.zeros(B * 128, np.int64)
        valid_node = np.zeros(B * 128, bool)
        slot = np.full(B * BLK_E, 127.0, np.float32)
        colv = np.zeros(B * BLK_E, np.int64)
        rowv = np.zeros(B * BLK_E, np.int64)
        evalid = np.zeros(B * BLK_E, bool)
        for b, (base, nn, es, ne) in enumerate(blocks):
            s = b * 128
            node_idx[s:s + nn] = np.arange(base, base + nn)
            valid_node[s:s + nn] = True
            t = b * BLK_E
            slot[t:t + ne] = (row_s[es:es + ne] - base).astype(np.float32)
            colv[t:t + ne] = col_s[es:es + ne]
            rowv[t:t + ne] = row_s[es:es + ne]
            evalid[t:t + ne] = True

        ev = evalid.astype(np.float32)
        crx = coord[rowv, 0] * ev
        cry = coord[rowv, 1] * ev
        ccx = coord[colv, 0] * ev
        ccy = coord[colv, 1] * ev
        vrx = vel[rowv, 0] * ev
        vry = vel[rowv, 1] * ev
        vcx = vel[colv, 0] * ev
        vcy = vel[colv, 1] * ev

        def tiles4(a0, a1, a2, a3):
            # [B*TPB, 4, TILE] from four flat [B*BLK_E] arrays
            return np.ascontiguousarray(
                np.stack([a.reshape(B * TPB, TILE) for a in (a0, a1, a2, a3)], axis=1)
            ).astype(np.float32)

        xvals = tiles4(crx, ccx, vrx, vcx)
        yvals = tiles4(cry, ccy, vry, vcy)
        cdiff = np.ascontiguousarray(
            np.stack([(crx - ccx).reshape(B * TPB, TILE),
                      (cry - ccy).reshape(B * TPB, TILE)], axis=1)
        ).astype(np.float32)
        cdflat = np.ascontiguousarray(cdiff.reshape(B * TPB, 1, 2 * TILE))
        slotc = np.ascontiguousarray(
            slot.reshape(B * TPB, 4, 128).transpose(0, 2, 1)).astype(np.float32)
        colidx = np.ascontiguousarray(
            colv.reshape(B * TPB, 4, 128).transpose(0, 2, 1)).astype(np.int32)
        slotr = np.ascontiguousarray(slot.reshape(B * TPB, TILE)).astype(np.float32)

        hT_blocks = np.ascontiguousarray(h[node_idx].T).astype(np.float32)
        # xy-flat per-block layout: [1, B*256], block b = [x(128) | y(128)]
        def xyflat(a):
            return np.ascontiguousarray(
                a[node_idx].reshape(B, 128, 2).transpose(0, 2, 1).reshape(1, B * 256)
            ).astype(np.float32)
        coordF = xyflat(coord)
        velF = xyflat(vel)

        per_core.append(dict(
            hT_blocks=hT_blocks, coordF=coordF, velF=velF,
            colidx=colidx, slotc=slotc, slotr=slotr,
            xvals=xvals, yvals=yvals, cdiff=cdiff, cdflat=cdflat,
        ))
        metas.append(dict(node_idx=node_idx, valid_node=valid_node))
    return per_core, metas, B


def _const_inputs(h, We1, be1, We2, be2, Wc1, bc1, Wc2,
                  Wn1, bn1, Wn2, bn2, Wv1, bv1, Wv2, bv2):
    f = np.float32
    Wrad = We1[2 * NF:2 * NF + 5]
    Wvr = We1[2 * NF + 5:]
    d = dict(
        h_full=np.ascontiguousarray(h, dtype=f),
        W_hr=np.ascontiguousarray(We1[0:NF], dtype=f),
        W_hc=np.ascontiguousarray(We1[NF:2 * NF], dtype=f),
        W_dd=np.ascontiguousarray(Wrad[4:5], dtype=f),
        Wx4=np.ascontiguousarray(np.stack([Wrad[0], Wrad[2], Wvr[0], Wvr[2]]), dtype=f),
        Wy4=np.ascontiguousarray(np.stack([Wrad[1], Wrad[3], Wvr[1], Wvr[3]]), dtype=f),
        Wy4n=np.ascontiguousarray(-np.stack([Wrad[1], Wrad[3], Wvr[1], Wvr[3]]), dtype=f),
        be1=np.ascontiguousarray(be1.reshape(H, 1), dtype=f),
        We2=np.ascontiguousarray(We2, dtype=f),
        be2=np.ascontiguousarray(be2.reshape(H, 1), dtype=f),
        Wc1q=np.ascontiguousarray(Wc1 * 0.25, dtype=f),
        bc1=np.ascontiguousarray(bc1.reshape(H, 1), dtype=f),
        Wc2=np.ascontiguousarray(Wc2, dtype=f),
        Wn1h=np.ascontiguousarray(Wn1[0:NF], dtype=f),
        Wn1a=np.ascontiguousarray(Wn1[NF:] * 0.25, dtype=f),
        bn1=np.ascontiguousarray(bn1.reshape(H, 1), dtype=f),
        Wn2=np.ascontiguousarray(Wn2, dtype=f),
        bn2=np.ascontiguousarray(bn2.reshape(NF, 1), dtype=f),
        Wv1=np.ascontiguousarray(Wv1, dtype=f),
        bv1=np.ascontiguousarray(bv1.reshape(H, 1), dtype=f),
        Wv2=np.ascontiguousarray(Wv2.reshape(H, 1), dtype=f),
        bv2=np.ascontiguousarray(np.asarray(bv2).reshape(1, 1), dtype=f),
        ident=np.eye(128, dtype=f),
        iota_p=np.arange(128, dtype=f).reshape(128, 1),
        iota_f=np.ascontiguousarray(np.tile(np.arange(128, dtype=f)[None, :], (128, 1))),
    )
    return d


def _build(B, n_nodes, num_devices=NCORES):
    import concourse.bass as bass
    import concourse.mybir as mybir
    import concourse.tile as tile
    from concourse import bacc

    dt = mybir.dt
    f32 = dt.float32
    AF = mybir.ActivationFunctionType
    OP = mybir.AluOpType

    nc = bacc.Bacc("TRN2", target_bir_lowering=False, debug=False,
                   num_devices=num_devices)

    def inp(name, shape, d=f32):
        return nc.dram_tensor(name, shape, d, kind="ExternalInput").ap()

    hD = inp("h_full", [n_nodes, NF])
    hTb = inp("hT_blocks", [128, B * 128])
    cFb = inp("coordF", [1, B * 256])
    vFb = inp("velF", [1, B * 256])
    colI = inp("colidx", [B * TPB, 128, 4], dt.int32)
    sltC = inp("slotc", [B * TPB, 128, 4])
    sltR = inp("slotr", [B * TPB, TILE])
    xvD = inp("xvals", [B * TPB, 4, TILE])
    yvD = inp("yvals", [B * TPB, 4, TILE])
    cdD = inp("cdiff", [B * TPB, 2, TILE])
    cdFD = inp("cdflat", [B * TPB, 1, 2 * TILE])
    wshapes = dict(W_hr=[NF, H], W_hc=[NF, H], W_dd=[1, H], Wx4=[4, H], Wy4=[4, H], Wy4n=[4, H],
                   be1=[H, 1], We2=[H, H], be2=[H, 1], Wc1q=[H, H], bc1=[H, 1],
                   Wc2=[H, 2], Wn1h=[NF, H], Wn1a=[H, H], bn1=[H, 1], Wn2=[H, NF],
                   bn2=[NF, 1], Wv1=[NF, H], bv1=[H, 1], Wv2=[H, 1], bv2=[1, 1],
                   ident=[128, 128], iota_p=[128, 1], iota_f=[128, 128])
    wD = {k: inp(k, s) for k, s in wshapes.items()}
    outH = nc.dram_tensor("out_hT", [128, B * 128], f32, kind="ExternalOutput").ap()
    outC = nc.dram_tensor("out_cF", [1, B * 256], f32, kind="ExternalOutput").ap()

    with tile.TileContext(nc) as tc:
        with ExitStack() as ctx:
            cpool = ctx.enter_context(tc.tile_pool(name="const", bufs=1))
            W = {}
            for k, s in wshapes.items():
                t = cpool.tile(s, f32, tag=k)
                nc.sync.dma_start(t[:], wD[k][:])
                W[k] = t
            ident = W["ident"]
            iota_p = W["iota_p"]
            iota_f = W["iota_f"]

            blk = ctx.enter_context(tc.tile_pool(name="blk", bufs=2))
            ep = ctx.enter_context(tc.tile_pool(name="edge", bufs=2))
            sp = ctx.enter_context(tc.tile_pool(name="scat", bufs=3))
            zp = ctx.enter_context(tc.tile_pool(name="z", bufs=4))
            ps_mlp = ctx.enter_context(tc.tile_pool(name="psmlp", bufs=3, space="PSUM"))
            ps_agg = ctx.enter_context(tc.tile_pool(name="psagg", bufs=1, space="PSUM"))
            ps_chk = ctx.enter_context(tc.tile_pool(name="pschk", bufs=2, space="PSUM"))
            ps_sml = ctx.enter_context(tc.tile_pool(name="pssml", bufs=2, space="PSUM"))

            for b in range(B):
                hT_b = blk.tile([128, 128], f32, tag="hT_b")
                nc.sync.dma_start(hT_b[:], hTb[:, b * 128:(b + 1) * 128])
                hwr_ps = ps_chk.tile([128, 128], f32, tag="c")
                nc.tensor.matmul(hwr_ps[:], hT_b[:], W["W_hr"][:], start=True, stop=True)
                hWr_b = blk.tile([128, 128], f32, tag="hWr")
                nc.scalar.copy(hWr_b[:], hwr_ps[:])

                agg_ps = ps_agg.tile([128, 131], f32, tag="agg")
                for tt in range(TPB):
                    t = b * TPB + tt
                    idx_t = ep.tile([128, 4], dt.int32, tag="idx")
                    nc.sync.dma_start(idx_t[:], colI[t])
                    sltc_t = ep.tile([128, 4], f32, tag="sltc")
                    nc.sync.dma_start(sltc_t[:], sltC[t])
                    sbc = ep.tile([128, TILE], f32, tag="sbc")
                    nc.sync.dma_start(sbc[:], sltR[t:t + 1, :].to_broadcast((128, TILE)))
                    xv = ep.tile([4, TILE], f32, tag="xv")
                    nc.sync.dma_start(xv[:], xvD[t])
                    yv = ep.tile([4, TILE], f32, tag="yv")
                    nc.sync.dma_start(yv[:], yvD[t])
                    cd = ep.tile([2, TILE], f32, tag="cd")
                    nc.sync.dma_start(cd[:], cdD[t])
                    cdf = ep.tile([1, 2 * TILE], f32, tag="cdf")
                    nc.sync.dma_start(cdf[:], cdFD[t])
                    gth = ep.tile([128, TILE], f32, tag="gth")
                    for c in range(4):
                        nc.gpsimd.indirect_dma_start(
                            out=gth[:, c * 128:(c + 1) * 128], out_offset=None,
                            in_=hD[:],
                            in_offset=bass.IndirectOffsetOnAxis(
                                ap=idx_t[:, c:c + 1], axis=0))

                    ST = ep.tile([128, TILE], f32, tag="ST")
                    nc.vector.tensor_scalar(ST[:], sbc[:], iota_p[:, 0:1], None, OP.is_equal)
                    S_all = ep.tile([128, TILE], f32, tag="S")
                    for c in range(4):
                        nc.vector.tensor_tensor(
                            S_all[:, c * 128:(c + 1) * 128],
                            sltc_t[:, c:c + 1].to_broadcast((128, 128)),
                            iota_f[:], OP.is_equal)
                    hcolT = ep.tile([128, TILE], f32, tag="hcolT")
                    for c in range(4):
                        tp = ps_chk.tile([128, 128], f32, tag="c")
                        nc.tensor.transpose(tp[:], gth[:, c * 128:(c + 1) * 128], ident[:])
                        nc.scalar.copy(hcolT[:, c * 128:(c + 1) * 128], tp[:])

                    sq = ep.tile([1, 2 * TILE], f32, tag="sq")
                    nc.gpsimd.tensor_tensor(sq[:], cdf[:], cdf[:], OP.mult)
                    ddt = ep.tile([1, TILE], f32, tag="dd")
                    nc.gpsimd.tensor_tensor(ddt[:], sq[:, 0:TILE], sq[:, TILE:2 * TILE],
                                            OP.add)

                    pre = ps_mlp.tile([128, TILE], f32, tag="m")
                    nc.tensor.matmul(pre[:], hWr_b[:], ST[:], start=True, stop=False)
                    nc.tensor.matmul(pre[:], W["W_hc"][:], hcolT[:], start=False, stop=False)
                    nc.tensor.matmul(pre[:], W["W_dd"][:], ddt[:], start=False, stop=True)
                    pre_sb = ep.tile([128, TILE], f32, tag="pre")
                    nc.scalar.copy(pre_sb[:], pre[:])
                    # U = X+Y, V = X-Y accumulated directly in PSUM (Wy4n = -Wy4)
                    U = ps_mlp.tile([128, TILE], f32, tag="m")
                    nc.tensor.matmul(U[:], W["Wx4"][:], xv[:], start=True, stop=False)
                    nc.tensor.matmul(U[:], W["Wy4"][:], yv[:], start=False, stop=True)
                    V = ps_mlp.tile([128, TILE], f32, tag="m")
                    nc.tensor.matmul(V[:], W["Wx4"][:], xv[:], start=True, stop=False)
                    nc.tensor.matmul(V[:], W["Wy4n"][:], yv[:], start=False, stop=True)

                    ef_sum = ep.tile([128, TILE], f32, tag="ef")
                    combos = [(U, OP.add), (U, OP.subtract), (V, OP.subtract), (V, OP.add)]
                    for g, (uv, op) in enumerate(combos):
                        z = zp.tile([128, TILE], f32, tag="z")
                        nc.vector.tensor_tensor(z[:], pre_sb[:], uv[:], op)
                        if g < 2:
                            nc.scalar.activation(z[:], z[:], AF.Relu, bias=W["be1"][:, 0:1])
                        else:
                            nc.vector.tensor_scalar(z[:], z[:], W["be1"][:, 0:1], 0.0,
                                                    OP.add, OP.max)
                        efp = ps_mlp.tile([128, TILE], f32, tag="m")
                        nc.tensor.matmul(efp[:], W["We2"][:], z[:], start=True, stop=True)
                        if g == 0:
                            nc.vector.tensor_scalar(ef_sum[:], efp[:], W["be2"][:, 0:1], 0.0,
                                                    OP.add, OP.max)
                        else:
                            tmp = zp.tile([128, TILE], f32, tag="eftmp")
                            if g < 3:
                                nc.scalar.activation(tmp[:], efp[:], AF.Relu,
                                                     bias=W["be2"][:, 0:1])
                            else:
                                nc.vector.tensor_scalar(tmp[:], efp[:], W["be2"][:, 0:1], 0.0,
                                                        OP.add, OP.max)
                            nc.gpsimd.tensor_tensor(ef_sum[:], ef_sum[:], tmp[:], OP.add)

                    q_ps = ps_mlp.tile([128, TILE], f32, tag="m")
                    nc.tensor.matmul(q_ps[:], W["Wc1q"][:], ef_sum[:], start=True, stop=True)
                    q_sb = ep.tile([128, TILE], f32, tag="q")
                    nc.scalar.activation(q_sb[:], q_ps[:], AF.Relu, bias=W["bc1"][:, 0:1])
                    cm_ps = ps_sml.tile([2, TILE], f32, tag="s")
                    nc.tensor.matmul(cm_ps[:], W["Wc2"][:], q_sb[:], start=True, stop=True)
                    tr = ep.tile([2, TILE], f32, tag="tr")
                    nc.vector.tensor_tensor(tr[:], cd[:], cm_ps[:], OP.mult)
                    trc = ep.tile([2, TILE], f32, tag="trc")
                    nc.gpsimd.tensor_scalar(trc[:], tr[:], 100.0, -100.0, OP.min, OP.max)

                    for c in range(4):
                        ftp = ps_chk.tile([128, 128], f32, tag="c")
                        nc.tensor.transpose(ftp[:], ef_sum[:, c * 128:(c + 1) * 128], ident[:])
                        scat = sp.tile([128, 131], f32, tag="scat")
                        nc.scalar.copy(scat[:, 0:128], ftp[:])
                        ttp = ps_sml.tile([128, 2], f32, tag="s")
                        nc.tensor.transpose(ttp[:], trc[:, c * 128:(c + 1) * 128],
                                            ident[0:2, 0:2])
                        nc.scalar.copy(scat[:, 128:130], ttp[:])
                        nc.gpsimd.memset(scat[:, 130:131], 1.0)
                        first = (tt == 0 and c == 0)
                        last = (tt == TPB - 1 and c == 3)
                        nc.tensor.matmul(agg_ps[:], S_all[:, c * 128:(c + 1) * 128], scat[:],
                                         start=first, stop=last)

                # ---- node phase ----
                agg_sb = blk.tile([128, 131], f32, tag="agg_sb")
                nc.scalar.copy(agg_sb[:], agg_ps[:])
                atp = ps_chk.tile([128, 128], f32, tag="c")
                nc.tensor.transpose(atp[:], agg_sb[:, 0:128], ident[:])
                aggT = blk.tile([128, 128], f32, tag="aggT")
                nc.scalar.copy(aggT[:], atp[:])
                # coord-agg x, y and cnt rows, each transposed to [1, 128] on p0
                acF = blk.tile([1, 256], f32, tag="acF")
                cntT = blk.tile([1, 128], f32, tag="cntT")
                for j, dst in ((0, acF[:, 0:128]), (1, acF[:, 128:256]), (2, cntT[:])):
                    rp = ps_sml.tile([1, 128], f32, tag="s")
                    nc.tensor.transpose(rp[:], agg_sb[:, 128 + j:129 + j], ident[:])
                    nc.vector.tensor_copy(dst, rp[:])

                n1p = ps_chk.tile([128, 128], f32, tag="c")
                nc.tensor.matmul(n1p[:], W["Wn1h"][:], hT_b[:], start=True, stop=False)
                nc.tensor.matmul(n1p[:], W["Wn1a"][:], aggT[:], start=False, stop=True)
                n1 = blk.tile([128, 128], f32, tag="n1")
                nc.scalar.activation(n1[:], n1p[:], AF.Relu, bias=W["bn1"][:, 0:1])
                o2p = ps_chk.tile([128, 128], f32, tag="c")
                nc.tensor.matmul(o2p[:], W["Wn2"][:], n1[:], start=True, stop=True)
                hn = blk.tile([128, 128], f32, tag="hn")
                nc.scalar.activation(hn[:], o2p[:], AF.Identity, bias=W["bn2"][:, 0:1])
                nc.vector.tensor_tensor(hn[:], hn[:], hT_b[:], OP.add)
                nc.sync.dma_start(outH[:, b * 128:(b + 1) * 128], hn[:])

                v1p = ps_chk.tile([128, 128], f32, tag="c")
                nc.tensor.matmul(v1p[:], W["Wv1"][:], hT_b[:], start=True, stop=True)
                v1 = blk.tile([128, 128], f32, tag="v1")
                nc.scalar.activation(v1[:], v1p[:], AF.Relu, bias=W["bv1"][:, 0:1])
                vsp = ps_sml.tile([1, 128], f32, tag="s")
                nc.tensor.matmul(vsp[:], W["Wv2"][:], v1[:], start=True, stop=True)
                vs = blk.tile([1, 128], f32, tag="vs")
                nc.scalar.activation(vs[:], vsp[:], AF.Identity, bias=W["bv2"][:, 0:1])

                coF = blk.tile([1, 256], f32, tag="coF")
                nc.sync.dma_start(coF[:], cFb[:, b * 256:(b + 1) * 256])
                veF = blk.tile([1, 256], f32, tag="veF")
                nc.sync.dma_start(veF[:], vFb[:, b * 256:(b + 1) * 256])
                cl = blk.tile([1, 128], f32, tag="cl")
                nc.vector.tensor_scalar(cl[:], cntT[:], 1.0, None, OP.max)
                rec = blk.tile([1, 128], f32, tag="rec")
                nc.vector.reciprocal(rec[:], cl[:])
                cnew = blk.tile([1, 256], f32, tag="cnew")
                for a in range(2):
                    sl = slice(a * 128, (a + 1) * 128)
                    nc.vector.tensor_tensor(cnew[:, sl], acF[:, sl], rec[:], OP.mult)
                    t2 = blk.tile([1, 128], f32, tag="t2")
                    nc.vector.tensor_tensor(t2[:], vs[:], veF[:, sl], OP.mult)
                    nc.vector.tensor_tensor(cnew[:, sl], cnew[:, sl], t2[:], OP.add)
                    nc.vector.tensor_tensor(cnew[:, sl], cnew[:, sl], coF[:, sl], OP.add)
                nc.sync.dma_start(outC[:, b * 256:(b + 1) * 256], cnew[:])

    nc.compile()
    return nc


def kernel(**inputs):
    h = np.asarray(inputs["h"], np.float32)
    coord = np.asarray(inputs["coord"], np.float32)
    vel = np.asarray(inputs["vel"], np.float32)
    edge_index = np.asarray(inputs["edge_index"])

    per_core, metas, B = _host_prep(h, coord, vel, edge_index)
    consts = _const_inputs(
        h, np.asarray(inputs["We1"], np.float32), np.asarray(inputs["be1"], np.float32),
        np.asarray(inputs["We2"], np.float32), np.asarray(inputs["be2"], np.float32),
        np.asarray(inputs["Wc1"], np.float32), np.asarray(inputs["bc1"], np.float32),
        np.asarray(inputs["Wc2"], np.float32),
        np.asarray(inputs["Wn1"], np.float32), np.asarray(inputs["bn1"], np.float32),
        np.asarray(inputs["Wn2"], np.float32), np.asarray(inputs["bn2"], np.float32),
        np.asarray(inputs["Wv1"], np.float32), np.asarray(inputs["bv1"], np.float32),
        np.asarray(inputs["Wv2"], np.float32), np.asarray(inputs["bv2"], np.float32))

    nc = _build(B, h.shape[0], num_devices=NCORES)
    in_maps = [dict(consts, **pc) for pc in per_core]

    from concourse.bass_utils import run_bass_kernel_spmd
    res = run_bass_kernel_spmd(nc, in_maps, core_ids=list(range(NCORES)))

    n = h.shape[0]
    h_new = np.empty((n, NF), np.float32)
    coord_new = np.empty((n, 2), np.float32)
    for c in range(NCORES):
        r = res.results[c]
        vi = metas[c]["valid_node"]
        ni = metas[c]["node_idx"]
        h_new[ni[vi]] = r["out_hT"].T[vi]
        cF = r["out_cF"].reshape(B, 2, 128).transpose(0, 2, 1).reshape(B * 128, 2)
        coord_new[ni[vi]] = cF[vi]
    return (h_new, coord_new)


# revision 17
# speedup vs baseline: 1.2098x; 1.2098x over previous
"""Trainium2 Bass kernel for nn_DE_GCL_vel_2D (GNN message passing, E(8e5) edges, N(5e4) nodes).

Strategy: sort edges by destination node (`row`), partition nodes into 8
contiguous ranges with ~equal edge counts -> each core's segment-sum is
purely local (no collective). Within a core, nodes are packed into blocks
of <=127 real nodes / <=2048 edges (slot 127 is a trash slot for padding
edges). Edge MLP runs feature-major on the PE; segment-sum is done with
one-hot matmuls accumulating into a per-block PSUM tile.
"""
import numpy as np
from contextlib import ExitStack

N = 50000
E = 800000
NF = 128
H = 128
NCORES = 8
TILE = 512           # edges per tile
TPB = 4              # tiles per block
BLK_E = TILE * TPB   # 2048 edges capacity per block
BLK_N = 127          # max real nodes per block (slot 127 = trash)


def _host_prep(h, coord, vel, edge_index):
    """Shard + marshal inputs. Returns (per_core_inputs, metas, B)."""
    row = np.asarray(edge_index[0]).astype(np.int64)
    col = np.asarray(edge_index[1]).astype(np.int64)
    n = h.shape[0]
    order = np.argsort(row, kind="stable")
    row_s = row[order]
    col_s = col[order]
    deg = np.bincount(row, minlength=n)
    cum = np.zeros(n + 1, np.int64)
    np.cumsum(deg, out=cum[1:])
    ne_total = row.shape[0]
    targets = (np.arange(1, NCORES) * ne_total) // NCORES
    inner = np.searchsorted(cum, targets)
    bounds = np.concatenate(([0], inner, [n])).astype(np.int64)
    assert deg.max() <= BLK_E, "node degree exceeds block capacity"

    cores_blocks = []
    for c in range(NCORES):
        n0, n1 = int(bounds[c]), int(bounds[c + 1])
        blocks = []
        nb = n0
        while nb < n1:
            base = nb
            ne = 0
            nn = 0
            while nb < n1 and nn < BLK_N and ne + deg[nb] <= BLK_E:
                ne += int(deg[nb])
                nn += 1
                nb += 1
            blocks.append((base, nn, int(cum[base]), ne))
        cores_blocks.append(blocks)
    B = max(len(bl) for bl in cores_blocks)

    per_core = []
    metas = []
    for c in range(NCORES):
        blocks = cores_blocks[c]
        node_idx = np.zeros(B * 128, np.int64)
        valid_node = np.zeros(B * 128, bool)
        slot = np.full(B * BLK_E, 127.0, np.float32)
        colv = np.zeros(B * BLK_E, np.int64)
        rowv = np.zeros(B * BLK_E, np.int64)
        evalid = np.zeros(B * BLK_E, bool)
        for b, (base, nn, es, ne) in enumerate(blocks):
            s = b * 128
            node_idx[s:s + nn] = np.arange(base, base + nn)
            valid_node[s:s + nn] = True
            t = b * BLK_E
            slot[t:t + ne] = (row_s[es:es + ne] - base).astype(np.float32)
            colv[t:t + ne] = col_s[es:es + ne]
            rowv[t:t + ne] = row_s[es:es + ne]
            evalid[t:t + ne] = True

        ev = evalid.astype(np.float32)
        crx = coord[rowv, 0] * ev
        cry = coord[rowv, 1] * ev
        ccx = coord[colv, 0] * ev
        ccy = coord[colv, 1] * ev
        vrx = vel[rowv, 0] * ev
        vry = vel[rowv, 1] * ev
        vcx = vel[colv, 0] * ev
        vcy = vel[colv, 1] * ev

        def tiles4(a0, a1, a2, a3):
            # [B*TPB, 4, TILE] from four flat [B*BLK_E] arrays
            return np.ascontiguousarray(
                np.stack([a.reshape(B * TPB, TILE) for a in (a0, a1, a2, a3)], axis=1)
            ).astype(np.float32)

        xv = tiles4(crx, ccx, vrx, vcx)
        yv = tiles4(cry, ccy, vry, vcy)
        xyvals = np.ascontiguousarray(np.concatenate([xv, yv], axis=1))
        cdiff = np.ascontiguousarray(
            np.stack([(crx - ccx).reshape(B * TPB, TILE),
                      (cry - ccy).reshape(B * TPB, TILE)], axis=1)
        ).astype(np.float32)
        slotc = np.ascontiguousarray(
            slot.reshape(B * TPB, 4, 128).transpose(0, 2, 1)).astype(np.float32)
        colidx = np.ascontiguousarray(
            colv.reshape(B * TPB, 4, 128).transpose(0, 2, 1)).astype(np.int32)
        # islot: cols 0:4 = col indices (int32), cols 4:8 = slot ids (f32 bits)
        islot = np.ascontiguousarray(np.concatenate(
            [colidx, slotc.view(np.int32)], axis=2))
        slotr = np.ascontiguousarray(slot.reshape(B * TPB, TILE)).astype(np.float32)

        hT_blocks = np.ascontiguousarray(h[node_idx].T).astype(np.float32)
        # xy-flat per-block layout: [1, B*256], block b = [x(128) | y(128)]
        def xyflat(a):
            return np.ascontiguousarray(
                a[node_idx].reshape(B, 128, 2).transpose(0, 2, 1).reshape(1, B * 256)
            ).astype(np.float32)
        coordF = xyflat(coord)
        velF = xyflat(vel)

        per_core.append(dict(
            hT_blocks=hT_blocks, coordF=coordF, velF=velF,
            islot=islot, slotr=slotr, xyvals=xyvals, cdiff=cdiff,
        ))
        metas.append(dict(node_idx=node_idx, valid_node=valid_node))
    return per_core, metas, B


def _const_inputs(h, We1, be1, We2, be2, Wc1, bc1, Wc2,
                  Wn1, bn1, Wn2, bn2, Wv1, bv1, Wv2, bv2):
    f = np.float32
    Wrad = We1[2 * NF:2 * NF + 5]
    Wvr = We1[2 * NF + 5:]
    d = dict(
        h_full=np.ascontiguousarray(h, dtype=f),
        W_hr=np.ascontiguousarray(We1[0:NF], dtype=f),
        W_hc=np.ascontiguousarray(We1[NF:2 * NF], dtype=f),
        Wdd2=np.ascontiguousarray(np.concatenate([Wrad[4:5], Wrad[4:5]]), dtype=f),
        WU8=np.ascontiguousarray(np.concatenate(
            [np.stack([Wrad[0], Wrad[2], Wvr[0], Wvr[2]]),
             np.stack([Wrad[1], Wrad[3], Wvr[1], Wvr[3]])]), dtype=f),
        WV8=np.ascontiguousarray(np.concatenate(
            [np.stack([Wrad[0], Wrad[2], Wvr[0], Wvr[2]]),
             -np.stack([Wrad[1], Wrad[3], Wvr[1], Wvr[3]])]), dtype=f),
        be1=np.ascontiguousarray(be1.reshape(H, 1), dtype=f),
        We2=np.ascontiguousarray(We2, dtype=f),
        be2=np.ascontiguousarray(be2.reshape(H, 1), dtype=f),
        Wc1q=np.ascontiguousarray(Wc1 * 0.25, dtype=f),
        bc1=np.ascontiguousarray(bc1.reshape(H, 1), dtype=f),
        Wc2=np.ascontiguousarray(Wc2, dtype=f),
        Wn1h=np.ascontiguousarray(Wn1[0:NF], dtype=f),
        Wn1a=np.ascontiguousarray(Wn1[NF:] * 0.25, dtype=f),
        bn1=np.ascontiguousarray(bn1.reshape(H, 1), dtype=f),
        Wn2=np.ascontiguousarray(Wn2, dtype=f),
        bn2=np.ascontiguousarray(bn2.reshape(NF, 1), dtype=f),
        Wv1=np.ascontiguousarray(Wv1, dtype=f),
        bv1=np.ascontiguousarray(bv1.reshape(H, 1), dtype=f),
        Wv2=np.ascontiguousarray(Wv2.reshape(H, 1), dtype=f),
        bv2=np.ascontiguousarray(np.asarray(bv2).reshape(1, 1), dtype=f),
        ident=np.eye(128, dtype=f),
        iota_p=np.arange(128, dtype=f).reshape(128, 1),
        iota_f=np.ascontiguousarray(np.tile(np.arange(128, dtype=f)[None, :], (128, 1))),
    )
    return d


def _build(B, n_nodes, num_devices=NCORES):
    import concourse.bass as bass
    import concourse.mybir as mybir
    import concourse.tile as tile
    from concourse import bacc

    dt = mybir.dt
    f32 = dt.float32
    AF = mybir.ActivationFunctionType
    OP = mybir.AluOpType

    nc = bacc.Bacc("TRN2", target_bir_lowering=False, debug=False,
                   num_devices=num_devices)

    def inp(name, shape, d=f32):
        return nc.dram_tensor(name, shape, d, kind="ExternalInput").ap()

    hD = inp("h_full", [n_nodes, NF])
    hTb = inp("hT_blocks", [128, B * 128])
    cFb = inp("coordF", [1, B * 256])
    vFb = inp("velF", [1, B * 256])
    islD = inp("islot", [B * TPB, 128, 8], dt.int32)
    sltR = inp("slotr", [B * TPB, TILE])
    xyD = inp("xyvals", [B * TPB, 8, TILE])
    cdD = inp("cdiff", [B * TPB, 2, TILE])
    wshapes = dict(W_hr=[NF, H], W_hc=[NF, H], Wdd2=[2, H], WU8=[8, H], WV8=[8, H],
                   be1=[H, 1], We2=[H, H], be2=[H, 1], Wc1q=[H, H], bc1=[H, 1],
                   Wc2=[H, 2], Wn1h=[NF, H], Wn1a=[H, H], bn1=[H, 1], Wn2=[H, NF],
                   bn2=[NF, 1], Wv1=[NF, H], bv1=[H, 1], Wv2=[H, 1], bv2=[1, 1],
                   ident=[128, 128], iota_p=[128, 1], iota_f=[128, 128])
    wD = {k: inp(k, s) for k, s in wshapes.items()}
    outH = nc.dram_tensor("out_hT", [128, B * 128], f32, kind="ExternalOutput").ap()
    outC = nc.dram_tensor("out_cF", [1, B * 256], f32, kind="ExternalOutput").ap()

    with tile.TileContext(nc) as tc:
        with ExitStack() as ctx:
            cpool = ctx.enter_context(tc.tile_pool(name="const", bufs=1))
            W = {}
            for k, s in wshapes.items():
                t = cpool.tile(s, f32, tag=k)
                nc.sync.dma_start(t[:], wD[k][:])
                W[k] = t
            ident = W["ident"]
            iota_p = W["iota_p"]
            iota_f = W["iota_f"]

            blk = ctx.enter_context(tc.tile_pool(name="blk", bufs=2))
            ep = ctx.enter_context(tc.tile_pool(name="edge", bufs=2))
            sp = ctx.enter_context(tc.tile_pool(name="scat", bufs=3))
            zp = ctx.enter_context(tc.tile_pool(name="z", bufs=4))
            ps_mlp = ctx.enter_context(tc.tile_pool(name="psmlp", bufs=3, space="PSUM"))
            ps_agg = ctx.enter_context(tc.tile_pool(name="psagg", bufs=1, space="PSUM"))
            ps_chk = ctx.enter_context(tc.tile_pool(name="pschk", bufs=2, space="PSUM"))
            ps_sml = ctx.enter_context(tc.tile_pool(name="pssml", bufs=2, space="PSUM"))

            for b in range(B):
                hT_b = blk.tile([128, 128], f32, tag="hT_b")
                nc.sync.dma_start(hT_b[:], hTb[:, b * 128:(b + 1) * 128])
                hwr_ps = ps_chk.tile([128, 128], f32, tag="c")
                nc.tensor.matmul(hwr_ps[:], hT_b[:], W["W_hr"][:], start=True, stop=True)
                hWr_b = blk.tile([128, 128], f32, tag="hWr")
                nc.scalar.copy(hWr_b[:], hwr_ps[:])

                agg_ps = ps_agg.tile([128, 131], f32, tag="agg")
                for tt in range(TPB):
                    t = b * TPB + tt
                    isl_t = ep.tile([128, 8], dt.int32, tag="isl")
                    nc.sync.dma_start(isl_t[:], islD[t])
                    sbc = ep.tile([128, TILE], f32, tag="sbc")
                    nc.sync.dma_start(sbc[:], sltR[t:t + 1, :].to_broadcast((128, TILE)))
                    xy = ep.tile([8, TILE], f32, tag="xy")
                    nc.sync.dma_start(xy[:], xyD[t])
                    cd = ep.tile([2, TILE], f32, tag="cd")
                    nc.sync.dma_start(cd[:], cdD[t])
                    gth = ep.tile([128, TILE], f32, tag="gth")
                    for c in range(4):
                        nc.gpsimd.indirect_dma_start(
                            out=gth[:, c * 128:(c + 1) * 128], out_offset=None,
                            in_=hD[:],
                            in_offset=bass.IndirectOffsetOnAxis(
                                ap=isl_t[:, c:c + 1], axis=0))

                    ST = ep.tile([128, TILE], f32, tag="ST")
                    nc.vector.tensor_scalar(ST[:], sbc[:], iota_p[:, 0:1], None, OP.is_equal)
                    S_all = ep.tile([128, TILE], f32, tag="S")
                    for c in range(4):
                        nc.vector.tensor_tensor(
                            S_all[:, c * 128:(c + 1) * 128],
                            isl_t[:, 4 + c:5 + c].bitcast(f32).to_broadcast((128, 128)),
                            iota_f[:], OP.is_equal)
                    hcolT = ep.tile([128, TILE], f32, tag="hcolT")
                    for c in range(4):
                        tp = ps_chk.tile([128, 128], f32, tag="c")
                        nc.tensor.transpose(tp[:], gth[:, c * 128:(c + 1) * 128], ident[:])
                        nc.scalar.copy(hcolT[:, c * 128:(c + 1) * 128], tp[:])

                    sq = ep.tile([2, TILE], f32, tag="sq")
                    nc.vector.tensor_tensor(sq[:], cd[:], cd[:], OP.mult)

                    pre = ps_mlp.tile([128, TILE], f32, tag="m")
                    nc.tensor.matmul(pre[:], hWr_b[:], ST[:], start=True, stop=False)
                    nc.tensor.matmul(pre[:], W["W_hc"][:], hcolT[:], start=False, stop=False)
                    nc.tensor.matmul(pre[:], W["Wdd2"][:], sq[:], start=False, stop=True)
                    pre_sb = ep.tile([128, TILE], f32, tag="pre")
                    nc.scalar.copy(pre_sb[:], pre[:])
                    # U = X+Y, V = X-Y accumulated directly in PSUM (WV8 has -Wy rows)
                    U = ps_mlp.tile([128, TILE], f32, tag="m")
                    nc.tensor.matmul(U[:], W["WU8"][:], xy[:], start=True, stop=True)
                    V = ps_mlp.tile([128, TILE], f32, tag="m")
                    nc.tensor.matmul(V[:], W["WV8"][:], xy[:], start=True, stop=True)

                    ef_sum = ep.tile([128, TILE], f32, tag="ef")
                    combos = [(U, OP.add), (U, OP.subtract), (V, OP.subtract), (V, OP.add)]
                    for g, (uv, op) in enumerate(combos):
                        z = zp.tile([128, TILE], f32, tag="z")
                        nc.vector.tensor_tensor(z[:], pre_sb[:], uv[:], op)
                        if g < 2:
                            nc.scalar.activation(z[:], z[:], AF.Relu, bias=W["be1"][:, 0:1])
                        else:
                            nc.vector.tensor_scalar(z[:], z[:], W["be1"][:, 0:1], 0.0,
                                                    OP.add, OP.max)
                        efp = ps_mlp.tile([128, TILE], f32, tag="m")
                        nc.tensor.matmul(efp[:], W["We2"][:], z[:], start=True, stop=True)
                        if g == 0:
                            nc.vector.tensor_scalar(ef_sum[:], efp[:], W["be2"][:, 0:1], 0.0,
                                                    OP.add, OP.max)
                        else:
                            tmp = zp.tile([128, TILE], f32, tag="eftmp")
                            nc.scalar.activation(tmp[:], efp[:], AF.Relu,
                                                 bias=W["be2"][:, 0:1])
                            eng = nc.gpsimd if g == 2 else nc.vector
                            eng.tensor_tensor(ef_sum[:], ef_sum[:], tmp[:], OP.add)

                    q_ps = ps_mlp.tile([128, TILE], f32, tag="m")
                    nc.tensor.matmul(q_ps[:], W["Wc1q"][:], ef_sum[:], start=True, stop=True)
                    q_sb = ep.tile([128, TILE], f32, tag="q")
                    nc.scalar.activation(q_sb[:], q_ps[:], AF.Relu, bias=W["bc1"][:, 0:1])
                    cm_ps = ps_sml.tile([2, TILE], f32, tag="s")
                    nc.tensor.matmul(cm_ps[:], W["Wc2"][:], q_sb[:], start=True, stop=True)
                    tr = ep.tile([2, TILE], f32, tag="tr")
                    nc.vector.tensor_tensor(tr[:], cd[:], cm_ps[:], OP.mult)
                    trc = ep.tile([2, TILE], f32, tag="trc")
                    nc.vector.tensor_scalar(trc[:], tr[:], 100.0, -100.0, OP.min, OP.max)

                    for c in range(4):
                        ftp = ps_chk.tile([128, 128], f32, tag="c")
                        nc.tensor.transpose(ftp[:], ef_sum[:, c * 128:(c + 1) * 128], ident[:])
                        scat = sp.tile([128, 131], f32, tag="scat")
                        nc.scalar.copy(scat[:, 0:128], ftp[:])
                        ttp = ps_sml.tile([128, 2], f32, tag="s")
                        nc.tensor.transpose(ttp[:], trc[:, c * 128:(c + 1) * 128],
                                            ident[0:2, 0:2])
                        nc.scalar.copy(scat[:, 128:130], ttp[:])
                        nc.vector.memset(scat[:, 130:131], 1.0)
                        first = (tt == 0 and c == 0)
                        last = (tt == TPB - 1 and c == 3)
                        nc.tensor.matmul(agg_ps[:], S_all[:, c * 128:(c + 1) * 128], scat[:],
                                         start=first, stop=last)

                # ---- node phase ----
                agg_sb = blk.tile([128, 131], f32, tag="agg_sb")
                nc.scalar.copy(agg_sb[:], agg_ps[:])
                atp = ps_chk.tile([128, 128], f32, tag="c")
                nc.tensor.transpose(atp[:], agg_sb[:, 0:128], ident[:])
                aggT = blk.tile([128, 128], f32, tag="aggT")
                nc.scalar.copy(aggT[:], atp[:])
                # coord-agg x, y and cnt rows, each transposed to [1, 128] on p0
                acF = blk.tile([1, 256], f32, tag="acF")
                cntT = blk.tile([1, 128], f32, tag="cntT")
                for j, dst in ((0, acF[:, 0:128]), (1, acF[:, 128:256]), (2, cntT[:])):
                    rp = ps_sml.tile([1, 128], f32, tag="s")
                    nc.tensor.transpose(rp[:], agg_sb[:, 128 + j:129 + j], ident[:])
                    nc.vector.tensor_copy(dst, rp[:])

                n1p = ps_chk.tile([128, 128], f32, tag="c")
                nc.tensor.matmul(n1p[:], W["Wn1h"][:], hT_b[:], start=True, stop=False)
                nc.tensor.matmul(n1p[:], W["Wn1a"][:], aggT[:], start=False, stop=True)
                n1 = blk.tile([128, 128], f32, tag="n1")
                nc.scalar.activation(n1[:], n1p[:], AF.Relu, bias=W["bn1"][:, 0:1])
                o2p = ps_chk.tile([128, 128], f32, tag="c")
                nc.tensor.matmul(o2p[:], W["Wn2"][:], n1[:], start=True, stop=True)
                hn = blk.tile([128, 128], f32, tag="hn")
                nc.scalar.activation(hn[:], o2p[:], AF.Identity, bias=W["bn2"][:, 0:1])
                nc.vector.tensor_tensor(hn[:], hn[:], hT_b[:], OP.add)
                nc.sync.dma_start(outH[:, b * 128:(b + 1) * 128], hn[:])

                v1p = ps_chk.tile([128, 128], f32, tag="c")
                nc.tensor.matmul(v1p[:], W["Wv1"][:], hT_b[:], start=True, stop=True)
                v1 = blk.tile([128, 128], f32, tag="v1")
                nc.scalar.activation(v1[:], v1p[:], AF.Relu, bias=W["bv1"][:, 0:1])
                vsp = ps_sml.tile([1, 128], f32, tag="s")
                nc.tensor.matmul(vsp[:], W["Wv2"][:], v1[:], start=True, stop=True)
                vs = blk.tile([1, 128], f32, tag="vs")
                nc.scalar.activation(vs[:], vsp[:], AF.Identity, bias=W["bv2"][:, 0:1])

                coF = blk.tile([1, 256], f32, tag="coF")
                nc.sync.dma_start(coF[:], cFb[:, b * 256:(b + 1) * 256])
                veF = blk.tile([1, 256], f32, tag="veF")
                nc.sync.dma_start(veF[:], vFb[:, b * 256:(b + 1) * 256])
                cl = blk.tile([1, 128], f32, tag="cl")
                nc.vector.tensor_scalar(cl[:], cntT[:], 1.0, None, OP.max)
                rec = blk.tile([1, 128], f32, tag="rec")
                nc.vector.reciprocal(rec[:], cl[:])
                cnew = blk.tile([1, 256], f32, tag="cnew")
                for a in range(2):
                    sl = slice(a * 128, (a + 1) * 128)
                    nc.vector.tensor_tensor(cnew[:, sl], acF[:, sl], rec[:], OP.mult)
                    t2 = blk.tile([1, 128], f32, tag="t2")
                    nc.vector.tensor_tensor(t2[:], vs[:], veF[:, sl], OP.mult)
                    nc.vector.tensor_tensor(cnew[:, sl], cnew[:, sl], t2[:], OP.add)
                    nc.vector.tensor_tensor(cnew[:, sl], cnew[:, sl], coF[:, sl], OP.add)
                nc.sync.dma_start(outC[:, b * 256:(b + 1) * 256], cnew[:])

    nc.compile()
    return nc


def kernel(**inputs):
    h = np.asarray(inputs["h"], np.float32)
    coord = np.asarray(inputs["coord"], np.float32)
    vel = np.asarray(inputs["vel"], np.float32)
    edge_index = np.asarray(inputs["edge_index"])

    per_core, metas, B = _host_prep(h, coord, vel, edge_index)
    consts = _const_inputs(
        h, np.asarray(inputs["We1"], np.float32), np.asarray(inputs["be1"], np.float32),
        np.asarray(inputs["We2"], np.float32), np.asarray(inputs["be2"], np.float32),
        np.asarray(inputs["Wc1"], np.float32), np.asarray(inputs["bc1"], np.float32),
        np.asarray(inputs["Wc2"], np.float32),
        np.asarray(inputs["Wn1"], np.float32), np.asarray(inputs["bn1"], np.float32),
        np.asarray(inputs["Wn2"], np.float32), np.asarray(inputs["bn2"], np.float32),
        np.asarray(inputs["Wv1"], np.float32), np.asarray(inputs["bv1"], np.float32),
        np.asarray(inputs["Wv2"], np.float32), np.asarray(inputs["bv2"], np.float32))

    nc = _build(B, h.shape[0], num_devices=NCORES)
    in_maps = [dict(consts, **pc) for pc in per_core]

    from concourse.bass_utils import run_bass_kernel_spmd
    res = run_bass_kernel_spmd(nc, in_maps, core_ids=list(range(NCORES)))

    n = h.shape[0]
    h_new = np.empty((n, NF), np.float32)
    coord_new = np.empty((n, 2), np.float32)
    for c in range(NCORES):
        r = res.results[c]
        vi = metas[c]["valid_node"]
        ni = metas[c]["node_idx"]
        h_new[ni[vi]] = r["out_hT"].T[vi]
        cF = r["out_cF"].reshape(B, 2, 128).transpose(0, 2, 1).reshape(B * 128, 2)
        coord_new[ni[vi]] = cF[vi]
    return (h_new, coord_new)
